# revision 5
# baseline (speedup 1.0000x reference)
"""Causal self-attention (GPT-style, B=2 T=4096 C=768 H=12) on 8 trn2 NeuronCores.

Sharding: data-parallel over batch (2) x tensor-parallel over head-groups (4):
core c handles batch c//4, heads 3*(c%4) .. 3*(c%4)+2. Each core computes
qkv projection, causal attention and its partial c_proj contribution; host
sums the 4 partials per batch and adds b_proj.

Device algorithm (per core, all matmuls fp32r = 1 cycle/row):
  - x^T [768,4096] is sharded on host (transpose is free there).
  - QK^T computed feature-major: 4 M-groups [q0|q1],[k0|k1],[q2|k2],[k2|q2]
    (the duplicate h2 layouts give base-partition-aligned lhsT/rhs pairs and
    alternate PE row-groups). V computed token-major with a fused
    ones-column so the AV matmul also produces softmax denominators.
  - Attention in S^T layout [k_tok, q_tok]: S^T block = K_blk^T.T @ Q^T tile,
    causal masks added on DVE (additive -1e30, diag blocks only), exp on ACT
    (scale=1/8 fused, 3 psum banks per call), AV accumulated in PSUM:
    O'^T[65,512] = sum_kb V'[kb].T @ P^T[kb]  (row 64 = softmax denom l).
  - normalize: r = 1/l (custom DVE fast reciprocal), partition-broadcast of r
    via SBUF->SBUF DMA, O^T = O'^T * r.
  - c_proj: y[tok,768] = sum_h O_h @ Wp_h, PSUM -> SBUF -> DMA out.
"""

import numpy as np

T = 4096
C = 768
HEADS = 12
HD = 64
HPC = 3          # heads per core
NCORES = 8
KS = C // 128    # 6 contraction subtiles
QT = 512         # query tile (psum bank width)
NQT = T // QT    # 8
KB = 128         # key block
NKB = T // KB    # 32
CHT = 512        # phase-A token chunk
NCH = T // CHT   # 8
NEG = -1.0e30

_NC_CACHE = {}


def _build_nc():
    import concourse.bacc as bacc
    import concourse.mybir as mybir
    import concourse.tile as tile

    F32 = mybir.dt.float32
    F32R = mybir.dt.float32r
    Exp = mybir.ActivationFunctionType.Exp

    nc = bacc.Bacc()

    xT_d = nc.declare_dram_parameter("xT", [C, T], F32R, isOutput=False)
    wqk_d = nc.declare_dram_parameter("wqk", [C, 512], F32R, isOutput=False)
    wv_d = nc.declare_dram_parameter("wv", [C, 256], F32R, isOutput=False)
    bqk_d = nc.declare_dram_parameter("bqk", [4, 128], F32, isOutput=False)
    bv_d = nc.declare_dram_parameter("bv", [128, 195], F32, isOutput=False)
    wp_d = nc.declare_dram_parameter("wp", [3, 64, 768], F32R, isOutput=False)
    mask_d = nc.declare_dram_parameter("masks", [4, 128, 512], F32, isOutput=False)
    ones_d = nc.declare_dram_parameter("ones", [128, 64], F32R, isOutput=False)
    y_d = nc.declare_dram_parameter("y", [T, C], F32, isOutput=True)

    xT_v = xT_d.rearrange("(ko ki) t -> ki ko t", ki=128)
    wqk_v = wqk_d.rearrange("(ko ki) m -> ki ko m", ki=128)
    wv_v = wv_d.rearrange("(ko ki) m -> ki ko m", ki=128)
    bqk_v = bqk_d.rearrange("g p -> p g")
    wp_v = wp_d.rearrange("h p n -> p h n")
    mask_v = mask_d.rearrange("m p q -> p m q")

    with tile.TileContext(nc) as tc:
        with (
            tc.tile_pool(name="singles", bufs=1) as singles,
            tc.tile_pool(name="xt", bufs=2) as xtp,
            tc.tile_pool(name="pt", bufs=2) as ptp,
            tc.tile_pool(name="o", bufs=2) as op_,
            tc.tile_pool(name="bc", bufs=2) as bcp,
            tc.tile_pool(name="yo", bufs=2) as yop,
            tc.tile_pool(name="sps", bufs=2, space="PSUM") as spool,
            tc.tile_pool(name="av", bufs=2, space="PSUM") as apool,
        ):
            wqk_sb = singles.tile([128, KS, 512], F32R)
            wv_sb = singles.tile([128, KS, 256], F32R)
            bqk_sb = singles.tile([128, 4], F32)
            bv_sb = singles.tile([128, 195], F32)
            wp_sb = singles.tile([64, 3, 768], F32R)
            mask_sb = singles.tile([128, 4, 512], F32)
            ones_sb = singles.tile([128, 64], F32R)
            nc.sync.dma_start(wqk_sb, wqk_v)
            nc.sync.dma_start(wv_sb, wv_v)
            nc.sync.dma_start(bqk_sb, bqk_v)
            nc.sync.dma_start(bv_sb, bv_d[:])
            nc.sync.dma_start(wp_sb, wp_v)
            nc.sync.dma_start(mask_sb, mask_v)
            nc.sync.dma_start(ones_sb, ones_d[:])

            # qk[g]: [128, T] feature-major tensors, g in 0..3:
            #   0: [q_h0; q_h1]  1: [k_h0; k_h1]  2: [q_h2; k_h2]  3: [k_h2; q_h2]
            qk_sb = [singles.tile([128, T], F32R, tag=f"qk{g}", name=f"qk{g}") for g in range(4)]
            # v: [tok128, kb, head, 65] with col 64 = 1.0 (from bias path)
            v_sb = singles.tile([128, NKB, HPC, 65], F32R)

            # ---------------- Phase A: qkv projection ----------------
            for ct in range(NCH):
                xt = xtp.tile([128, KS, CHT], F32R)
                nc.sync.dma_start(xt, xT_v[:, :, ct * CHT:(ct + 1) * CHT])
                for g in range(4):
                    ps = spool.tile([128, 3, QT], F32, tag="sps")
                    for ks in range(KS):
                        nc.tensor.matmul(
                            ps[:, 0, :],
                            wqk_sb[:, ks, 128 * g:128 * (g + 1)],
                            xt[:, ks, :],
                            start=(ks == 0),
                            stop=(ks == KS - 1),
                        )
                    nc.scalar.add(
                        out=qk_sb[g][:, ct * CHT:(ct + 1) * CHT],
                        in_=ps[:, 0, :],
                        add=bqk_sb[:, g:g + 1],
                    )
                for tt in range(4):
                    kb = ct * 4 + tt
                    vps = apool.tile([128, QT], F32, tag="av")
                    for ks in range(KS):
                        nc.tensor.matmul(
                            vps[:, 0:256],
                            xt[:, ks, tt * 128:(tt + 1) * 128],
                            wv_sb[:, ks, :],
                            start=(ks == 0),
                            stop=(ks == KS - 1),
                        )
                    nc.vector.tensor_add(
                        out=v_sb[:, kb, :, :],
                        in0=vps[:, 0:195].rearrange("p (h d) -> p h d", h=3),
                        in1=bv_sb.rearrange("p (h d) -> p h d", h=3),
                    )

            # ---------------- Phase B: attention + proj ----------------
            def q_ap(h, qt):
                qs = slice(qt * QT, (qt + 1) * QT)
                if h == 0:
                    return qk_sb[0][0:64, qs]
                if h == 1:
                    return qk_sb[0][64:128, qs]
                return None  # h2 handled separately (alternating)

            def attention_pass(qt, entries, avps, n_kb):
                """entries: list of (h, kb). avps: {h: psum tile}."""
                for c0 in range(0, len(entries), 3):
                    chunk = entries[c0:c0 + 3]
                    ln = len(chunk)
                    sps = spool.tile([128, 3, QT], F32, tag="sps")
                    for j, (h, kb) in enumerate(chunk):
                        kbs = slice(kb * KB, (kb + 1) * KB)
                        qs = slice(qt * QT, (qt + 1) * QT)
                        if h == 0:
                            lhsT, rhs = qk_sb[1][0:64, kbs], qk_sb[0][0:64, qs]
                        elif h == 1:
                            lhsT, rhs = qk_sb[1][64:128, kbs], qk_sb[0][64:128, qs]
                        elif kb % 2 == 0:
                            lhsT, rhs = qk_sb[3][0:64, kbs], qk_sb[2][0:64, qs]
                        else:
                            lhsT, rhs = qk_sb[2][64:128, kbs], qk_sb[3][64:128, qs]
                        nc.tensor.matmul(sps[:, j, :], lhsT, rhs, start=True, stop=True)
                    for j, (h, kb) in enumerate(chunk):
                        m = kb - 4 * qt
                        if m >= 0:
                            w = (m + 1) * 128
                            nc.vector.tensor_add(
                                out=sps[:, j, 0:w],
                                in0=sps[:, j, 0:w],
                                in1=mask_sb[:, m, 0:w],
                            )
                    pt = ptp.tile([128, 3, QT], F32R)
                    nc.scalar.activation(
                        out=pt[:, 0:ln, :], in_=sps[:, 0:ln, :], func=Exp, scale=0.125
                    )
                    for j, (h, kb) in enumerate(chunk):
                        nc.tensor.matmul(
                            avps[h][0:65, :],
                            v_sb[:, kb, h, :],
                            pt[:, j, :],
                            start=(kb == 0),
                            stop=(kb == n_kb - 1),
                        )

            def normalize(avp, o_dst):
                lsb = bcp.tile([65, QT], F32R, tag="rt")
                nc.vector.tensor_copy(lsb[64:65, :], avp[64:65, :])
                bc_ps = spool.tile([128, 3, QT], F32, tag="sps")
                nc.tensor.matmul(
                    bc_ps[0:64, 0, :], ones_sb[64:65, :], lsb[64:65, :],
                    start=True, stop=True,
                )
                rb = bcp.tile([64, QT], F32, tag="bc")
                nc.vector.reciprocal_approx_fast(out=rb, in_=bc_ps[0:64, 0, :])
                nc.vector.tensor_mul(out=o_dst, in0=avp[0:64, :], in1=rb)

            for qt in range(NQT):
                n_kb = 4 * qt + 4
                o_t = [op_.tile([64, QT], F32R, tag=f"o{h}", name=f"o{h}") for h in range(HPC)]

                av01 = {h: apool.tile([128, QT], F32, tag="av", name=f"av{h}") for h in (0, 1)}
                entries = [(h, kb) for kb in range(n_kb) for h in (0, 1)]
                attention_pass(qt, entries, av01, n_kb)
                normalize(av01[0], o_t[0])
                normalize(av01[1], o_t[1])

                av2 = {2: apool.tile([128, QT], F32, tag="av", name="av2")}
                attention_pass(qt, [(2, kb) for kb in range(n_kb)], av2, n_kb)
                normalize(av2[2], o_t[2])

                for mtt in range(4):
                    msl = slice(mtt * 128, (mtt + 1) * 128)
                    pp = spool.tile([128, 768], F32, tag="sps")
                    for nchunk in ((0, 512), (512, 768)):
                        n0, n1 = nchunk
                        for h in range(HPC):
                            nc.tensor.matmul(
                                pp[:, n0:n1],
                                o_t[h][:, msl],
                                wp_sb[:, h, n0:n1],
                                start=(h == 0),
                                stop=(h == HPC - 1),
                            )
                    yt = yop.tile([128, 768], F32)
                    nc.vector.tensor_copy(yt, pp)
                    nc.sync.dma_start(
                        y_d[qt * QT + mtt * 128: qt * QT + (mtt + 1) * 128, :], yt
                    )

    nc.finalize()
    return nc


def _get_nc():
    if "nc" not in _NC_CACHE:
        _NC_CACHE["nc"] = _build_nc()
    return _NC_CACHE["nc"]


def _shard_inputs(x, W_attn, b_attn, W_proj):
    """Build the 8 per-core input maps."""
    in_maps = []
    qcol = lambda h: slice(64 * h, 64 * h + 64)
    kcol = lambda h: slice(C + 64 * h, C + 64 * h + 64)
    vcol = lambda h: slice(2 * C + 64 * h, 2 * C + 64 * h + 64)

    # causal additive masks: mask[m, k', q'] = NEG where q' < 128*m + k'
    kk = np.arange(128)[:, None]
    qq = np.arange(512)[None, :]
    masks = np.zeros((4, 128, 512), dtype=np.float32)
    for m in range(4):
        masks[m] = np.where(qq < 128 * m + kk, NEG, 0.0).astype(np.float32)

    for core in range(NCORES):
        b, hg = divmod(core, 4)
        hs = [3 * hg, 3 * hg + 1, 3 * hg + 2]

        xT = np.ascontiguousarray(x[b].T)  # [C, T]

        wqk = np.empty((C, 512), dtype=np.float32)
        bqk = np.empty((4, 128), dtype=np.float32)
        groups = [
            (qcol(hs[0]), qcol(hs[1])),
            (kcol(hs[0]), kcol(hs[1])),
            (qcol(hs[2]), kcol(hs[2])),
            (kcol(hs[2]), qcol(hs[2])),
        ]
        for g, (c1, c2) in enumerate(groups):
            wqk[:, 128 * g:128 * g + 64] = W_attn[:, c1]
            wqk[:, 128 * g + 64:128 * g + 128] = W_attn[:, c2]
            bqk[g, 0:64] = b_attn[c1]
            bqk[g, 64:128] = b_attn[c2]

        wv = np.zeros((C, 256), dtype=np.float32)
        bv = np.zeros((128, 195), dtype=np.float32)
        for i, h in enumerate(hs):
            wv[:, 65 * i:65 * i + 64] = W_attn[:, vcol(h)]
            bv[:, 65 * i:65 * i + 64] = b_attn[vcol(h)][None, :]
            bv[:, 65 * i + 64] = 1.0

        wp = np.empty((3, 64, 768), dtype=np.float32)
        for i, h in enumerate(hs):
            wp[i] = W_proj[64 * h:64 * h + 64, :]

        in_maps.append(
            {
                "xT": xT,
                "wqk": wqk,
                "wv": wv,
                "bqk": bqk,
                "bv": bv,
                "wp": np.ascontiguousarray(wp),
                "masks": masks,
                "ones": np.ones((128, 64), dtype=np.float32),
            }
        )
    return in_maps


def kernel(x, W_attn, b_attn, W_proj, b_proj, _trace=False):
    from concourse.bass_utils import run_bass_kernel_spmd

    x = np.asarray(x, dtype=np.float32)
    W_attn = np.asarray(W_attn, dtype=np.float32)
    b_attn = np.asarray(b_attn, dtype=np.float32)
    W_proj = np.asarray(W_proj, dtype=np.float32)
    b_proj = np.asarray(b_proj, dtype=np.float32)

    nc = _get_nc()
    in_maps = _shard_inputs(x, W_attn, b_attn, W_proj)
    res = run_bass_kernel_spmd(
        nc, in_maps, core_ids=list(range(NCORES)), trace=_trace
    )
    _NC_CACHE["last_result"] = res

    B = x.shape[0]
    y = np.empty((B, T, C), dtype=np.float32)
    for b in range(B):
        acc = res.results[4 * b + 0]["y"].astype(np.float32).copy()
        for hg in range(1, 4):
            acc += res.results[4 * b + hg]["y"]
        y[b] = acc + b_proj[None, :]
    return y
